# revision 14
# baseline (speedup 1.0000x reference)
"""Causal self-attention with RoPE on 8 trn2 NeuronCores.

Sharding: heads 2r,2r+1 -> core r (both batches). w_attn column-sharded
(rows permuted even/odd per head so interleaved RoPE becomes rotate-half);
attention computed per-core in transposed [tk, tq] score layout; one
AllToAll per local head re-shards heads->sequence so each core runs the
full output projection for its own 512-token slice. Host only
slices/permutes/casts inputs and concatenates the 8 output slices.
Matmul operands are bf16 (fp32 PSUM accumulation); fp32 elsewhere.

PSUM layout: two tags TK/TQ, each a [128,2,512] 2-bank pair tile with
bufs=2 (8 banks total), shared by every phase:
  phase1  pk[i]/pq[i] qkv pairs; v pairs on recycled slots (k-groups run
          first so k-rope frees TK slots before the v pass needs them)
  phase2  score pairs on TK; o+rowsums fused in one TQ pair tile
  phase3  projection accumulators, one per t-subtile
"""

import math
import os
import sys
import tempfile

if "/opt/trn_rl_repo" not in sys.path:
    sys.path.insert(0, "/opt/trn_rl_repo")

import ml_dtypes
import numpy as np

import concourse.bacc as bacc
import concourse.bass as bass
import concourse.mybir as mybir
import concourse.tile as tile
from concourse.bass_utils import run_bass_kernel_spmd

B, T, D = 2, 2048, 2048
H, HD = 16, 128
NCORES = 8
HL = H // NCORES          # heads per core
BT = B * T
TS = 512                  # t supertile (psum bank width in fp32)
NTB = T // TS             # supertiles per batch
NE = D // 128             # e-chunks (contraction) in qkv
NKC = T // 128            # tk chunks per batch
FQKV = 3 * HL * HD        # 768 qkv features per core
FP = mybir.dt.float32
BF = mybir.dt.bfloat16
SCALE = 1.0 / math.sqrt(HD)

LAST_EXEC_NS = None
LAST_TRACE = None

_built = {}


def _install_ntff_shim():
    import types

    import antenv

    if "antenv.axon_hooks" not in sys.modules:
        mod = types.ModuleType("antenv.axon_hooks")
        _hook = [None]
        mod.set_axon_ntff_profile_hook = lambda h: _hook.__setitem__(0, h)
        mod.get_axon_ntff_profile_hook = lambda: _hook[0]
        sys.modules["antenv.axon_hooks"] = mod
        antenv.axon_hooks = mod
    from antenv.axon_hooks import (
        get_axon_ntff_profile_hook,
        set_axon_ntff_profile_hook,
    )

    if get_axon_ntff_profile_hook() is None:
        from trn_agent_boot.trn_boot import _ntff_profile_via_ctypes

        set_axon_ntff_profile_hook(_ntff_profile_via_ctypes("/opt/axon/libaxon_pjrt.so"))
    import concourse.bass_utils as bu

    bu.upload_artifacts = lambda tmpdir: f"local:{tmpdir}"


def _build():
    if "nc" in _built:
        return _built["nc"]
    nc = bacc.Bacc("TRN2", target_bir_lowering=False, debug=False, num_devices=NCORES)

    xT = nc.dram_tensor("xT", [D, BT], BF, kind="ExternalInput")
    wT = nc.dram_tensor("wT", [D, FQKV], BF, kind="ExternalInput")
    wpT = nc.dram_tensor("wpT", [D, D], BF, kind="ExternalInput")
    cs2 = nc.dram_tensor("cs2", [128, T], FP, kind="ExternalInput")
    sn2 = nc.dram_tensor("sn2", [128, T], FP, kind="ExternalInput")
    out_loc = nc.dram_tensor("out_loc", [TS, D], FP, kind="ExternalOutput")

    from contextlib import ExitStack

    with tile.TileContext(nc) as tc:
        with ExitStack() as whole:
            dpool = whole.enter_context(tc.tile_pool(name="dram", bufs=1, space="DRAM"))
            a2a_in = [
                dpool.tile([NCORES, HD, TS], BF, name=f"a2a_in{hl}")
                for hl in range(HL)
            ]
            a2a_out = [
                dpool.tile([NCORES, HD, TS], BF, name=f"a2a_out{hl}")
                for hl in range(HL)
            ]
            psum = whole.enter_context(tc.tile_pool(name="psum", bufs=1, space="PSUM"))
            smallp = whole.enter_context(tc.tile_pool(name="small", bufs=1))
            ones_f = smallp.tile([128, 1], FP, name="ones_f")
            nc.vector.memset(ones_f[:], 1.0)
            ones = smallp.tile([128, 1], BF, name="ones")
            nc.vector.tensor_copy(out=ones[:], in_=ones_f[:])

            wq = whole.enter_context(tc.tile_pool(name="wq", bufs=1))
            w_sb = wq.tile([128, NE, FQKV], BF, name="w_sb")
            for c8 in range(8):
                nc.sync.dma_start(
                    out=w_sb[:, c8 * 2 : (c8 + 1) * 2, :],
                    in_=wT[c8 * 256 : (c8 + 1) * 256, :].rearrange(
                        "(c p) f -> p c f", p=128
                    ),
                )

            tabs = whole.enter_context(tc.tile_pool(name="tabs", bufs=1))
            cs_sb = tabs.tile([128, T], FP, name="cs_sb")
            sn_sb = tabs.tile([128, T], FP, name="sn_sb")
            nc.sync.dma_start(out=cs_sb[:], in_=cs2[:])
            nc.sync.dma_start(out=sn_sb[:], in_=sn2[:])

            store = whole.enter_context(tc.tile_pool(name="store", bufs=1))
            xtp = whole.enter_context(tc.tile_pool(name="xt", bufs=3))
            work = whole.enter_context(tc.tile_pool(name="work", bufs=1))
            exps = whole.enter_context(tc.tile_pool(name="exps", bufs=4))
            osbp = whole.enter_context(tc.tile_pool(name="osb", bufs=2))
            bcp = whole.enter_context(tc.tile_pool(name="bc", bufs=2))
            recp = whole.enter_context(tc.tile_pool(name="rec", bufs=2))
            yp = whole.enter_context(tc.tile_pool(name="yp", bufs=1))
            wpp = whole.enter_context(tc.tile_pool(name="wpp", bufs=8))
            outp = whole.enter_context(tc.tile_pool(name="outp", bufs=2))

            def rope(dst, pe_, po_, b, tb, gen):
                csl = cs_sb[:, tb * TS : (tb + 1) * TS]
                snl = sn_sb[:, tb * TS : (tb + 1) * TS]
                a_ = work.tile([128, TS], FP, tag="w0", name=f"a_{b}_{tb}_{gen}")
                b_ = work.tile([128, TS], FP, tag="w1", name=f"b_{b}_{tb}_{gen}")
                c_ = work.tile([128, TS], FP, tag="w2", name=f"c_{b}_{tb}_{gen}")
                d_ = work.tile([128, TS], FP, tag="w3", name=f"d_{b}_{tb}_{gen}")
                nc.vector.tensor_tensor(a_[:], pe_, csl, mybir.AluOpType.mult)
                nc.vector.tensor_tensor(b_[:], po_, snl, mybir.AluOpType.mult)
                nc.vector.tensor_tensor(c_[:], pe_, snl, mybir.AluOpType.mult)
                nc.vector.tensor_tensor(d_[:], po_, csl, mybir.AluOpType.mult)
                tsl = slice(tb * TS, (tb + 1) * TS)
                for hl in range(HL):
                    hs = slice(hl * 64, (hl + 1) * 64)
                    nc.vector.tensor_tensor(
                        dst[hl][0:64, tsl], a_[hs, :], b_[hs, :],
                        mybir.AluOpType.subtract,
                    )
                    nc.vector.tensor_tensor(
                        dst[hl][64:128, tsl], c_[hs, :], d_[hs, :],
                        mybir.AluOpType.add,
                    )

            qrots, krots, v_alls = {}, {}, {}
            for b in range(B):
                qrot = [
                    store.tile([128, T], BF, tag=f"qrot{hl}_{b}", name=f"qrot{hl}_{b}")
                    for hl in range(HL)
                ]
                krot = [
                    store.tile([128, T], BF, tag=f"krot{hl}_{b}", name=f"krot{hl}_{b}")
                    for hl in range(HL)
                ]
                v_all = store.tile(
                    [128, HL, NKC, HD], BF, tag=f"v_all_{b}", name=f"v_all_{b}"
                )
                qrots[b], krots[b], v_alls[b] = qrot, krot, v_all

                # ---- phase 1: qkv projection + rope, tb-supertile pairs ----
                # k groups run first so k-rope frees TK slots before the v pass.
                for tbp in range(NTB // 2):
                    tbs = (2 * tbp, 2 * tbp + 1)
                    xts = []
                    for i, tb in enumerate(tbs):
                        toff = b * T + tb * TS
                        xt_t = xtp.tile(
                            [128, NE, TS], BF, tag="xt", name=f"xt_{b}_{tb}"
                        )
                        for eh in range(2):
                            nc.gpsimd.dma_start(
                                out=xt_t[:, eh * 8 : (eh + 1) * 8, :],
                                in_=xT[
                                    eh * 1024 : (eh + 1) * 1024, toff : toff + TS
                                ].rearrange("(c p) t -> p c t", p=128),
                            )
                        xts.append(xt_t)
                    pk = [
                        psum.tile(
                            [128, 2, TS], FP, tag="TK", bufs=2, name=f"pk{i}_{b}_{tbp}"
                        )
                        for i in range(2)
                    ]
                    pq = [
                        psum.tile(
                            [128, 2, TS], FP, tag="TQ", bufs=2, name=f"pq{i}_{b}_{tbp}"
                        )
                        for i in range(2)
                    ]
                    # group order: k_even, k_odd, q_even, q_odd (w cols per host layout)
                    GROUPS = [
                        (256, lambda i: pk[i][:, 0, :]),
                        (384, lambda i: pk[i][:, 1, :]),
                        (0, lambda i: pq[i][:, 0, :]),
                        (128, lambda i: pq[i][:, 1, :]),
                    ]
                    for g, (off, dst) in enumerate(GROUPS):
                        for e in range(NE):
                            for i in range(2):
                                nc.tensor.matmul(
                                    dst(i),
                                    lhsT=w_sb[:, e, off : off + 128],
                                    rhs=xts[i][:, e, :],
                                    start=(e == 0),
                                    stop=(e == NE - 1),
                                    skip_group_check=True,
                                )
                        if g == 1:
                            for i, tb in enumerate(tbs):
                                rope(krot, pk[i][:, 0, :], pk[i][:, 1, :], b, tb, "k")
                        if g == 3:
                            for i, tb in enumerate(tbs):
                                rope(qrot, pq[i][:, 0, :], pq[i][:, 1, :], b, tb, "q")
                    # v pass, st-major; slots freed by k-rope first, then q-rope
                    pv = [
                        psum.tile(
                            [128, 2, 2 * HD], FP, tag="TK" if st < 2 else "TQ",
                            bufs=2, padded_shape=[128, 2, TS],
                            name=f"pv{st}_{b}_{tbp}",
                        )
                        for st in range(4)
                    ]
                    for st in range(4):
                        for e in range(NE):
                            for i in range(2):
                                nc.tensor.matmul(
                                    pv[st][:, i, :],
                                    lhsT=xts[i][:, e, st * 128 : (st + 1) * 128],
                                    rhs=w_sb[:, e, 512:768],
                                    start=(e == 0),
                                    stop=(e == NE - 1),
                                    skip_group_check=True,
                                )
                    for st in range(4):
                        for i, tb in enumerate(tbs):
                            j = tb * 4 + st
                            for hl in range(HL):
                                nc.vector.tensor_copy(
                                    out=v_all[:, hl, j, :],
                                    in_=pv[st][:, i, hl * HD : (hl + 1) * HD],
                                )

            # ---- phase 2: attention, hl-outer; one all-to-all per hl ----
            y = [None] * NE
            for hl in range(HL):
                for b in range(B):
                    qrot, krot, v_all = qrots[b], krots[b], v_alls[b]
                    for tb in range(NTB):
                        # fused accumulator: bank 0 = attention out, bank 1 row 0 = exp sums
                        po_ = psum.tile(
                            [128, 2, TS], FP, tag="TQ", bufs=2, name=f"o_{b}_{hl}_{tb}"
                        )
                        nj = 4 * tb + 4
                        sp = None
                        for j in range(nj):
                            jh = j % 2
                            if jh == 0:
                                sp = psum.tile(
                                    [128, 2, TS], FP, tag="TK", bufs=2,
                                    name=f"s_{b}_{hl}_{tb}_{j}",
                                )
                            nc.tensor.matmul(
                                sp[:, jh, :],
                                lhsT=krot[hl][:, j * 128 : (j + 1) * 128],
                                rhs=qrot[hl][:, tb * TS : (tb + 1) * TS],
                                start=True,
                                stop=True,
                                skip_group_check=True,
                            )
                            ex = exps.tile(
                                [128, TS], BF, tag="exp", name=f"e_{b}_{hl}_{tb}_{j}"
                            )
                            nc.scalar.activation(
                                out=ex[:], in_=sp[:, jh, :],
                                func=mybir.ActivationFunctionType.Exp, scale=SCALE,
                            )
                            if j >= 4 * tb:
                                nc.gpsimd.affine_select(
                                    out=ex[:], in_=ex[:],
                                    pattern=[[1, TS]],
                                    compare_op=mybir.AluOpType.is_ge,
                                    fill=0.0,
                                    base=tb * TS - j * 128,
                                    channel_multiplier=-1,
                                )
                            nc.tensor.matmul(
                                po_[:, 0, :],
                                lhsT=v_all[:, hl, j, :],
                                rhs=ex[:],
                                start=(j == 0),
                                stop=(j == nj - 1),
                                skip_group_check=True,
                            )
                            nc.tensor.matmul(
                                po_[:1, 1, :],
                                lhsT=ones[:],
                                rhs=ex[:],
                                start=(j == 0),
                                stop=(j == nj - 1),
                                skip_group_check=True,
                            )
                        rec = recp.tile([1, TS], FP, tag="rec", name=f"r_{b}_{hl}_{tb}")
                        nc.vector.reciprocal_approx_fast(out=rec[:], in_=po_[:1, 1, :])
                        bc = bcp.tile([128, TS], FP, tag="bc", name=f"bc_{b}_{hl}_{tb}")
                        nc.gpsimd.partition_broadcast(bc[:], rec[:])
                        osb = osbp.tile([128, TS], BF, tag="osb", name=f"ot_{b}_{hl}_{tb}")
                        nc.vector.tensor_tensor(
                            osb[:], po_[:, 0, :], bc[:], mybir.AluOpType.mult
                        )
                        nc.sync.dma_start(
                            out=a2a_in[hl][b * NTB + tb, :, :],
                            in_=osb[:],
                        )
                # heads->sequence re-shard for this hl (overlaps next hl's attention)
                nc.gpsimd.collective_compute(
                    "AllToAll",
                    mybir.AluOpType.bypass,
                    replica_groups=[list(range(NCORES))],
                    ins=[a2a_in[hl][:].opt()],
                    outs=[a2a_out[hl][:].opt()],
                )
                for src in range(NCORES):
                    ci = HL * src + hl
                    yt = yp.tile([128, TS], BF, tag=f"y{ci}", name=f"y{ci}")
                    nc.sync.dma_start(out=yt[:], in_=a2a_out[hl][src, :, :])
                    y[ci] = yt

            # ---- phase 3: output projection for the local 512-token slice ----
            for dq in range(4):
                pp = [
                    psum.tile(
                        [128, TS], FP, tag="TK" if t2 < 2 else "TQ", bufs=2,
                        name=f"pp{t2}_{dq}",
                    )
                    for t2 in range(4)
                ]
                for ci in range(NE):
                    wpt = wpp.tile([128, TS], BF, tag="wp", name=f"wp_{dq}_{ci}")
                    nc.sync.dma_start(
                        out=wpt[:],
                        in_=wpT[ci * 128 : (ci + 1) * 128, dq * TS : (dq + 1) * TS],
                    )
                    for t2 in range(4):
                        nc.tensor.matmul(
                            pp[t2][:],
                            lhsT=y[ci][:, t2 * 128 : (t2 + 1) * 128],
                            rhs=wpt[:],
                            start=(ci == 0),
                            stop=(ci == NE - 1),
                            skip_group_check=True,
                        )
                for t2 in range(4):
                    ob = outp.tile([128, TS], FP, tag="ob", name=f"ob_{dq}_{t2}")
                    nc.vector.tensor_copy(out=ob[:], in_=pp[t2][:])
                    nc.sync.dma_start(
                        out=out_loc[t2 * 128 : (t2 + 1) * 128, dq * TS : (dq + 1) * TS],
                        in_=ob[:],
                    )

    nc.compile()
    _built["nc"] = nc
    return nc


def _host_prep(x, w_attn, w_proj):
    bf = ml_dtypes.bfloat16
    x2 = np.ascontiguousarray(x.reshape(BT, D).T.astype(bf))  # [D, BT] e-major
    wpT_full = np.ascontiguousarray(w_proj.T.astype(bf))      # [c, d]

    inv = 1.0 / (10000.0 ** (np.arange(0, HD, 2, dtype=np.float32) / HD))
    t = np.arange(T, dtype=np.float32)
    fr = np.outer(t, inv)                          # [T, 64]
    cosT = np.cos(fr).T.astype(np.float32)         # [64, T]
    sinT = np.sin(fr).T.astype(np.float32)
    cs2v = np.ascontiguousarray(np.vstack([cosT, cosT]))
    sn2v = np.ascontiguousarray(np.vstack([sinT, sinT]))

    perm = np.concatenate([np.arange(0, HD, 2), np.arange(1, HD, 2)])
    in_maps = []
    for r in range(NCORES):
        h0, h1 = HL * r, HL * r + 1
        rows = []
        for off in (0, D):  # q block then k block
            rows += [off + h0 * HD + perm[:64], off + h1 * HD + perm[:64]]
            rows += [off + h0 * HD + perm[64:], off + h1 * HD + perm[64:]]
        rows += [2 * D + h0 * HD + np.arange(HD), 2 * D + h1 * HD + np.arange(HD)]
        w_c = w_attn[np.concatenate(rows)]         # [768, D]
        wT_c = np.ascontiguousarray(w_c.T.astype(bf))  # [D, 768]
        in_maps.append(
            {"xT": x2, "wT": wT_c, "wpT": wpT_full, "cs2": cs2v, "sn2": sn2v}
        )
    return in_maps


def kernel(x, w_attn, w_proj):
    global LAST_EXEC_NS, LAST_TRACE
    x = np.asarray(x, dtype=np.float32)
    w_attn = np.asarray(w_attn, dtype=np.float32)
    w_proj = np.asarray(w_proj, dtype=np.float32)

    trace = os.environ.get("KERNEL_TRACE") == "1"
    if trace:
        _install_ntff_shim()

    nc = _build()
    in_maps = _host_prep(x, w_attn, w_proj)
    kw = {}
    if trace:
        tmpdir = os.environ.get("KERNEL_TRACE_DIR") or tempfile.mkdtemp(prefix="ktrace_")
        kw = dict(trace=True, tmpdir=tmpdir)
        LAST_TRACE = tmpdir
    res = run_bass_kernel_spmd(nc, in_maps, list(range(NCORES)), **kw)
    LAST_EXEC_NS = res.exec_time_ns

    out = np.empty((B, T, D), dtype=np.float32)
    for r in range(NCORES):
        b, tb = divmod(r, NTB)
        out[b, tb * TS : (tb + 1) * TS, :] = res.results[r]["out_loc"]
    return out
